# revision 31
# baseline (speedup 1.0000x reference)
"""DiscreteBipartiteFlow forward on 8 trn2 NeuronCores.

Math: inputs rows are exact one-hots (x0|x1). net = relu(x0@W1+b1)@W2+b2
only depends on i0=argmax(x0), so precompute on device the [V, 2V] table
NET = relu(W1+b1)@W2+b2 and its per-row argmaxes L[i], S[i]. For one-hot
x1 with index a1, z1 = one_hot((L[i0] + a1*S[i0]) mod V) (zero row when
S[i0]==0, scale index 0 being excluded). Output = [x0 | z1].

Per-core structure (1024 rows, 8 rows per partition):
 - ALL loads on the SP ring in priority order (w1t chunk0 + biases
   first so relu/NET start ASAP, then w2 per-k, then x) — the 16 DMA
   engines are shared between rings, so ring order decides arrival.
   Stores alternate SP/ACT rings.
 - b2 folded into PSUM via a ones-row matmul heading the NET
   accumulation; full-width (N=2V) fp32 chunk matmuls; argmax
   (max8/max_index) reads PSUM directly.
 - ONE combined lookup table in PSUM [P, 2V]: cols 0:V hold
   pk[v] = 128*S + 16384*L + 2^21*[S==0] (TWO accumulating bf16
   matmuls — both addends exactly representable in bf16 — with
   lhsT = per-partition-replicated values, rhs = identity), cols V:2V
   hold iota (ones-row x iota-row matmul, runs at kernel start). ONE
   256-wide dot per row-slot (scalar_tensor_tensor accum) over the
   full input row then yields comb = pk[i0] + a1 exactly in fp32.
 - int unpack (dual-op shifts/masks, multiply on GpSimd), z1 =
   is_equal(iota, c) written into a [P,8,2V] out tile whose x0 half the
   ACT engine copied; four [P,2,2V] stores alternate SP/ACT rings.
Data-parallel over 8 cores; weights replicated (host marshalling only).
"""

import numpy as np

V = 128
H = 512
N_CORES = 8
P = 128
NJ = 8               # row slots per partition
KH = H // P          # 4 contraction chunks

# host-marshalled weight buffer layout (fp32 columns per partition)
W1K0_OFF = 0         # [P, 128]  w1t k0
B1_OFF = 128         # [P, 4]    b1[p, k] = b1[128k+p]
B2_OFF = 132         # [P, 256]  b2 (used on partition 0 only)
W1R_OFF = 388        # [P, 384]  w1t k1..k3
W2_OFF = 772         # [P, 1024] w2[p, k*256+c] = W2[128k+p, c]
WB_COLS = 1796


def build_bass(rows: int):
    """Build the single-core Bass program for a [rows, 2V] batch shard."""
    import concourse.bacc as bacc
    import concourse.bass as bass
    import concourse.tile as tile
    from concourse import mybir

    f32 = mybir.dt.float32
    bf16 = mybir.dt.bfloat16
    i32 = mybir.dt.int32
    u32 = mybir.dt.uint32
    A = mybir.AluOpType
    AF = mybir.ActivationFunctionType

    assert rows == P * NJ

    nc = bacc.Bacc(None)
    x = nc.declare_dram_parameter("x", [rows, 2 * V], f32, isOutput=False)
    wb = nc.declare_dram_parameter("wb", [P, WB_COLS], f32, isOutput=False)
    out = nc.declare_dram_parameter("out", [rows, 2 * V], f32, isOutput=True)

    x_r = x.rearrange("(p j) n -> p j n", j=NJ)
    out_r = out.rearrange("(p j) n -> p j n", j=NJ)

    def bcast_mid(t_ap, reps):
        return bass.AP(
            tensor=t_ap.tensor, offset=t_ap.offset,
            ap=[t_ap.ap[0], [0, reps]] + list(t_ap.ap[1:]),
        )

    def bcast_last(t_ap, reps):
        return bass.AP(
            tensor=t_ap.tensor, offset=t_ap.offset,
            ap=list(t_ap.ap) + [[0, reps]],
        )

    with tile.TileContext(nc) as tc:
        with (
            tc.tile_pool(name="main", bufs=1) as main,
            tc.tile_pool(name="psum_net", bufs=1, space="PSUM") as psum_net,
            tc.tile_pool(name="psum_pb", bufs=1, space="PSUM") as psum_pb,
        ):
            # ---- loads, all SP ring, priority order ----
            wb_sb = main.tile([P, WB_COLS], f32)
            nc.sync.dma_start(out=wb_sb[:, 0:W1R_OFF], in_=wb[:, 0:W1R_OFF])
            nc.sync.dma_start(out=wb_sb[:, W2_OFF:W2_OFF + 2 * V],
                              in_=wb[:, W2_OFF:W2_OFF + 2 * V])
            nc.sync.dma_start(out=wb_sb[:, W1R_OFF:W2_OFF], in_=wb[:, W1R_OFF:W2_OFF])
            for k in range(1, KH):
                o = W2_OFF + k * 2 * V
                nc.sync.dma_start(out=wb_sb[:, o:o + 2 * V], in_=wb[:, o:o + 2 * V])
            xw = main.tile([P, NJ, 2 * V], f32)
            nc.sync.dma_start(out=xw[:, 0:4, :], in_=x_r[:, 0:4, :])
            nc.sync.dma_start(out=xw[:, 4:8, :], in_=x_r[:, 4:8, :])

            b1s = wb_sb[:, B1_OFF:B1_OFF + KH]
            b2r = wb_sb[0:1, B2_OFF:B2_OFF + 2 * V]
            w2 = wb_sb[:, W2_OFF:W2_OFF + KH * 2 * V].rearrange("p (k c) -> p k c", k=KH)

            def w1tk(k):
                if k == 0:
                    return wb_sb[:, W1K0_OFF:W1K0_OFF + V]
                o = W1R_OFF + (k - 1) * V
                return wb_sb[:, o:o + V]

            # ---- on-device constants ----
            iota_f = main.tile([P, V], f32)
            nc.gpsimd.iota(iota_f, pattern=[[1, V]], base=0, channel_multiplier=0,
                           allow_small_or_imprecise_dtypes=True)
            ipart_f = main.tile([P, 1], f32)
            nc.gpsimd.iota(ipart_f, pattern=[[0, 1]], base=0, channel_multiplier=1,
                           allow_small_or_imprecise_dtypes=True)
            ident_bf = main.tile([P, V], bf16)
            nc.vector.tensor_tensor(out=ident_bf, in0=iota_f,
                                    in1=bcast_last(ipart_f, V), op=A.is_equal)
            ones1 = main.tile([1, V], f32)
            nc.vector.memset(ones1, 1.0)

            # combined lookup table, iota half (constants only, runs early)
            packB = psum_pb.tile([P, 2 * V], f32)
            nc.tensor.matmul(packB[:, V:2 * V], lhsT=ones1, rhs=iota_f[0:1, :],
                             start=True, stop=True)

            # ---- table phase: full-width fp32 chunks, b2 heads the group --
            hT = main.tile([P, KH, V], f32)
            for k in range(KH):
                nc.scalar.activation(out=hT[:, k, :], in_=w1tk(k), func=AF.Relu,
                                     bias=b1s[:, k:k + 1], scale=1.0)
            net_ps = psum_net.tile([P, 2 * V], f32)
            nc.tensor.matmul(net_ps, lhsT=ones1, rhs=b2r, start=True, stop=False)
            for k in range(KH):
                nc.tensor.matmul(net_ps, lhsT=hT[:, k, :], rhs=w2[:, k, :],
                                 start=False, stop=(k == KH - 1))

            # ---- argmax per head straight off PSUM (S head first) ----
            idx = []
            for head in (1, 0):
                m8 = main.tile([P, 8], f32, tag=f"m8{head}")
                nc.vector.max(m8, net_ps[:, head * V:(head + 1) * V])
                ix = main.tile([P, 8], u32, tag=f"ix{head}")
                nc.vector.max_index(ix, m8, net_ps[:, head * V:(head + 1) * V])
                idx.append(ix)
                if head == 1:
                    # scale-side scalars straight off the u32 indices:
                    # pkS128 = 128*S (exact in bf16); z2M = 2^21*[S==0]
                    # pre-scaled so the post-L path is a single fused op
                    z2M = main.tile([P, 1], f32)
                    nc.vector.tensor_scalar(out=z2M, in0=ix[:, 0:1], scalar1=0.5,
                                            scalar2=2097152.0, op0=A.is_lt,
                                            op1=A.mult)
                    pkS128 = main.tile([P, V], bf16)
                    nc.vector.tensor_scalar(out=pkS128,
                                            in0=bcast_last(ix[:, 0:1], V),
                                            scalar1=float(V), scalar2=None,
                                            op0=A.mult)
            idxS, idxL = idx

            # L-side: pk_hi = 16384*L + 2^21*[S==0] (exact in bf16)
            pkhi_rep = main.tile([P, V], bf16)
            nc.vector.tensor_scalar(out=pkhi_rep, in0=bcast_last(idxL[:, 0:1], V),
                                    scalar1=float(V * V), scalar2=z2M[:, 0:1],
                                    op0=A.mult, op1=A.add)

            # pk half of the lookup table via two accumulating bf16 matmuls
            nc.tensor.matmul(packB[:, 0:V], lhsT=pkS128, rhs=ident_bf,
                             start=True, stop=False)
            nc.tensor.matmul(packB[:, 0:V], lhsT=pkhi_rep, rhs=ident_bf,
                             start=False, stop=True)

            # ---- x0 passthrough into the output tile (ACT, hidden) ----
            out_sb = main.tile([P, NJ, 2 * V], f32)
            for h in range(2):
                nc.scalar.copy(out_sb[:, 4 * h:4 * h + 4, 0:V],
                               xw[:, 4 * h:4 * h + 4, 0:V])

            # ---- 256-wide dots: comb[p,j] = pk[i0] + a1, exact in fp32 ----
            dot_scr = main.tile([P, NJ, 2 * V], f32)
            combf = main.tile([P, NJ], f32)
            for j in range(NJ):
                nc.vector.scalar_tensor_tensor(
                    out=dot_scr[:, j, :], in0=xw[:, j, :], scalar=1.0,
                    in1=packB, op0=A.bypass, op1=A.mult,
                    accum_out=combf[:, j:j + 1],
                )

            # ---- unpack: c = ((L + a1*S) & 127) | 128*[S==0] ----
            ci = main.tile([P, NJ], i32)
            nc.vector.tensor_copy(ci, combf)
            a1x = main.tile([P, NJ], i32)
            nc.vector.tensor_scalar(out=a1x, in0=ci, scalar1=V - 1, scalar2=None,
                                    op0=A.bitwise_and)
            sx = main.tile([P, NJ], i32)
            nc.vector.tensor_scalar(out=sx, in0=ci, scalar1=7, scalar2=V - 1,
                                    op0=A.arith_shift_right, op1=A.bitwise_and)
            sa = main.tile([P, NJ], i32)
            nc.gpsimd.tensor_mul(sa, a1x, sx)
            li = main.tile([P, NJ], i32)
            nc.vector.tensor_scalar(out=li, in0=ci, scalar1=14, scalar2=V - 1,
                                    op0=A.arith_shift_right, op1=A.bitwise_and)
            zi = main.tile([P, NJ], i32)
            nc.vector.tensor_scalar(out=zi, in0=ci, scalar1=14, scalar2=V,
                                    op0=A.arith_shift_right, op1=A.bitwise_and)
            t2 = main.tile([P, NJ], i32)
            nc.vector.tensor_add(t2, sa, li)
            c0 = main.tile([P, NJ], i32)
            nc.vector.tensor_scalar(out=c0, in0=t2, scalar1=V - 1, scalar2=None,
                                    op0=A.bitwise_and)
            cc = main.tile([P, NJ], i32)
            nc.vector.tensor_tensor(out=cc, in0=c0, in1=zi, op=A.bitwise_or)
            cf = main.tile([P, NJ], f32)
            nc.vector.tensor_copy(cf, cc)

            # ---- z1 into out_sb + store per 2-slot chunk (SP/ACT rings) ----
            for ch in range(4):
                js = ch * 2
                nc.vector.tensor_tensor(
                    out=out_sb[:, js:js + 2, V:2 * V],
                    in0=bcast_mid(iota_f, 2),
                    in1=bcast_last(cf[:, js:js + 2], V),
                    op=A.is_equal,
                )
                eng = nc.sync if ch % 2 == 0 else nc.scalar
                eng.dma_start(out=out_r[:, js:js + 2, :], in_=out_sb[:, js:js + 2, :])

    nc.finalize()
    return nc


# Test-harness hooks: extra kwargs for run_bass_kernel_spmd (e.g. trace=True)
# and the last BassKernelResults for profiling. Unused when graded.
RUN_KWARGS: dict = {}
LAST_RESULTS = None


def kernel(**inputs) -> np.ndarray:
    global LAST_RESULTS
    from concourse.bass_utils import run_bass_kernel_spmd

    x = np.ascontiguousarray(np.asarray(inputs["inputs"], dtype=np.float32))
    W1 = np.asarray(inputs["W1"], dtype=np.float32)
    b1 = np.asarray(inputs["b1"], dtype=np.float32)
    W2 = np.asarray(inputs["W2"], dtype=np.float32)
    b2 = np.asarray(inputs["b2"], dtype=np.float32)

    # pure layout marshalling into one per-partition weight buffer
    w1t = W1.T.reshape(KH, P, V).transpose(1, 0, 2)      # [P, KH, V]
    wb = np.zeros((P, WB_COLS), np.float32)
    wb[:, W1K0_OFF:W1K0_OFF + V] = w1t[:, 0, :]
    wb[:, B1_OFF:B1_OFF + KH] = b1.reshape(KH, P).T
    wb[:, B2_OFF:B2_OFF + 2 * V] = b2.reshape(1, 2 * V)
    wb[:, W1R_OFF:W1R_OFF + 3 * V] = w1t[:, 1:, :].reshape(P, 3 * V)
    wb[:, W2_OFF:W2_OFF + KH * 2 * V] = (
        W2.reshape(KH, P, 2 * V).transpose(1, 0, 2).reshape(P, KH * 2 * V)
    )

    B = x.shape[0]
    rows = B // N_CORES
    nc = build_bass(rows)

    shards = np.split(x, N_CORES, axis=0)
    in_maps = [{"x": s, "wb": wb} for s in shards]
    res = run_bass_kernel_spmd(nc, in_maps, list(range(N_CORES)), **RUN_KWARGS)
    LAST_RESULTS = res
    return np.concatenate([r["out"] for r in res.results], axis=0)
